# revision 23
# baseline (speedup 1.0000x reference)
"""Bass/Tile kernel for nn_CrossModalIntegrationLayer on 8 trn2 NeuronCores.

Sharding (SPMD-identical program; all per-core differences live in host-prepped
input data):
  Launch 1: core c -> (b = c//2, dhalf = c%2). The DI (d_inner) axis is
  host-permuted per core so that the core's own 512 scan channels are always
  channel-blocks 0..3; blocks 4..7 (the other half) are computed only as far as
  needed (conv -> content_proj). Each core runs LN1, in_proj, dwconv+silu,
  style/content projections, the 4-direction selective scan for blocks 0..3,
  y*z and its out_proj partial. Host sums the two partials per batch.
  Launch 2: core c -> 512 tokens of the flattened (B*L) axis: residual, LN2,
  MLP, residual.

Engine budget for the scan phase (the dominant cost, K*N*4db = 1024 units of
[128, 1024]):
  ACT:  decay = exp(A*delta) per unit                   (~1038 ns/unit)
  DVE:  all scans (~1127) + dbu = du*bcast(B) TTs (~594)
  Pool: most hC = h*bcast(C) multiplies (TT ~2127; walrus rejects scans/STT
        on Pool, so plain TensorTensor is the only Pool-legal form)
  PE:   y += I @ hC accumulated in PSUM over n
Everything hot is bf16 (2x DVE mode needs 2-byte packed operands); weights are
host-converted to bf16 for all large matmuls.
"""
import numpy as np
import ml_dtypes
from contextlib import ExitStack

import concourse.bass as bass
import concourse.tile as tile
from concourse import mybir

dt = mybir.dt
AF = mybir.ActivationFunctionType
ALU = mybir.AluOpType
MS = bass.MemorySpace
BF16 = ml_dtypes.bfloat16

HID, DI, N, R, K, Bb, MLP = 512, 1024, 64, 32, 4, 4, 2048
Hh = Ww = 32
L = Hh * Ww
P = 128
F2 = (slice(0, 512), slice(512, 1024))

L1_INPUTS = [
    ("cT", [HID, L], dt.float32), ("sT", [HID, L], dt.float32),
    ("w_inproj", [HID, DI + 512], dt.bfloat16),
    ("w_conv", [P, 72], dt.float32), ("conv_b", [P, 8], dt.float32),
    ("w_styleproj", [HID, K * (R + N)], dt.bfloat16),
    ("w_contentproj", [DI, K * N], dt.bfloat16),
    ("w_dt", [32, 4 * 512], dt.float32), ("dt_bias", [P, 16], dt.float32),
    ("A_logs_sb", [P, 1024], dt.float32), ("Ds_sb", [P, 16], dt.float32),
    ("w_outproj", [HID, HID], dt.bfloat16),
    ("n1w", [P, 4], dt.float32), ("n1b", [P, 4], dt.float32),
]
L2_INPUTS = [
    ("oT", [HID, 512], dt.float32), ("rT", [HID, 512], dt.float32),
    ("w_mlp1", [HID, MLP], dt.bfloat16), ("b_mlp1", [P, 16], dt.float32),
    ("w_mlp2", [MLP, HID], dt.bfloat16), ("b_mlp2", [P, 4], dt.float32),
    ("n2w", [P, 4], dt.float32), ("n2b", [P, 4], dt.float32),
]

# hC-split tuning: walrus only allows plain TensorTensor on Pool, and scans
# only on DVE. DVE carries all scans + all dbu; Pool carries most hC
# multiplies (2127 ns vs DVE 594 ns, but DVE is saturated by scans).
def _dve_hc(k, n, db):
    return (n * 4 + db) % 6 == 5


def _layer_norm16(nc, stage, lnps, x_tiles, out_tiles, nw, nb, ones, T=L):
    """LayerNorm over the partition (feature) axis of 4 [128, T] fp32 tiles,
    writing bf16 normalized output tiles."""
    fsl = [slice(i, min(i + 512, T)) for i in range(0, T, 512)]
    mps = lnps.tile([1, T], dt.float32, tag="ln_ps")
    sps = lnps.tile([1, T], dt.float32, tag="ln_ps2")
    for hi in range(4):
        sq = stage.tile([P, T], dt.float32, bufs=1, tag="ln_sq")
        nc.scalar.square(sq[:], x_tiles[hi][:])
        for f in fsl:
            nc.tensor.matmul(mps[:, f], ones[:, 0:1], x_tiles[hi][:, f],
                             start=(hi == 0), stop=(hi == 3))
            nc.tensor.matmul(sps[:, f], ones[:, 1:2], sq[:, f],
                             start=(hi == 0), stop=(hi == 3))
    inv_n = 1.0 / HID
    m_row = stage.tile([1, T], dt.float32, bufs=1, tag="ln_m")
    nc.scalar.activation(m_row[:], mps[:], AF.Copy, scale=inv_n)
    msq = stage.tile([1, T], dt.float32, bufs=1, tag="ln_msq")
    nc.scalar.square(msq[:], m_row[:])
    var = stage.tile([1, T], dt.float32, bufs=1, tag="ln_var")
    nc.scalar.activation(var[:], sps[:], AF.Copy, scale=inv_n)
    nc.vector.tensor_tensor(var[:], var[:], msq[:], ALU.subtract)
    eps = stage.tile([1, 1], dt.float32, bufs=1, tag="ln_eps")
    nc.vector.memset(eps[:], 1e-5)
    std = stage.tile([1, T], dt.float32, bufs=1, tag="ln_std")
    nc.scalar.activation(std[:], var[:], AF.Sqrt, bias=eps[:])
    rstd = stage.tile([1, T], dt.float32, bufs=1, tag="ln_rstd")
    nc.vector.reciprocal(rstd[:], std[:])
    m_bc = stage.tile([P, T], dt.float32, bufs=1, tag="ln_mbc")
    r_bc = stage.tile([P, T], dt.float32, bufs=1, tag="ln_rbc")
    nc.gpsimd.partition_broadcast(m_bc[:], m_row[:])
    nc.gpsimd.partition_broadcast(r_bc[:], rstd[:])
    for hi in range(4):
        t16 = stage.tile([P, T], dt.bfloat16, bufs=2, tag="ln_t16")
        nc.vector.scalar_tensor_tensor(t16[:], x_tiles[hi][:], 1.0, m_bc[:],
                                       ALU.mult, ALU.subtract)
        nc.vector.tensor_tensor(t16[:], t16[:], r_bc[:], ALU.mult)
        nc.vector.tensor_scalar(out_tiles[hi][:], t16[:],
                                nw[:, hi:hi + 1], nb[:, hi:hi + 1],
                                ALU.mult, ALU.add)


def _u_view(xa_tile, k):
    """Direction-k view (over the free/token axis) of an xa [128, 1024] tile."""
    t = xa_tile[:]
    if k == 0:
        return t, False
    if k == 2:
        return t[:, ::-1], False
    t3 = t.rearrange("p (h w) -> p h w", h=Hh).rearrange("p h w -> p w h")
    if k == 1:
        return t3, True
    return t3[:, ::-1, ::-1], True


def build_launch1(nc, dbg=()):
    din = {name: nc.dram_tensor(name, shape, dty, kind="ExternalInput").ap()
           for name, shape, dty in L1_INPUTS}
    opart = nc.dram_tensor("opart", [HID, L], dt.float32, kind="ExternalOutput").ap()
    dbg_names = []

    def emit_dbg(name, tiles):
        if name not in dbg:
            return
        dty = tiles[0].dtype if hasattr(tiles[0], "dtype") else dt.float32
        arr = nc.dram_tensor("dbg_" + name, [len(tiles) * P, L], dty,
                             kind="ExternalOutput").ap()
        for i, t in enumerate(tiles):
            nc.sync.dma_start(arr[i * P:(i + 1) * P, :], t[:])
        dbg_names.append(name)

    ctx = ExitStack()
    with ctx:
        tc = ctx.enter_context(tile.TileContext(nc, trace_sim=False))
        wpool = ctx.enter_context(tc.tile_pool(name="wpool", bufs=1))
        persist = ctx.enter_context(tc.tile_pool(name="persist", bufs=1))

        def load(pool, name, bufs_tag=None):
            src = din[name]
            nrows, fs = src.shape
            dty = src.tensor.dtype
            tiles = []
            for i in range(0, nrows, P):
                r = min(P, nrows - i)
                t = pool.tile([r, fs], dty, bufs=1,
                              tag=bufs_tag or f"w_{name}_{i}", name=f"ld_{name}_{i}")
                nc.sync.dma_start(t[:], src[i:i + r, :])
                tiles.append(t)
            return tiles

        ones = persist.tile([P, 2], dt.float32)
        nc.vector.memset(ones[:], 1.0)
        iota_t = persist.tile([P, P], dt.float32)
        nc.gpsimd.iota(iota_t[:], pattern=[[1, P]], base=0, channel_multiplier=-1,
                       allow_small_or_imprecise_dtypes=True)
        identity = persist.tile([P, P], dt.float32)
        nc.vector.tensor_scalar(identity[:], iota_t[:], 0.0, None, ALU.is_equal)
        ident16 = persist.tile([P, P], dt.bfloat16)
        nc.vector.tensor_copy(ident16[:], identity[:])
        A_sb = persist.tile([P, 1024], dt.float32)
        nc.sync.dma_start(A_sb[:], din["A_logs_sb"][:])
        nc.scalar.activation(A_sb[:], A_sb[:], AF.Exp)
        nc.vector.tensor_scalar(A_sb[:], A_sb[:], -1.0, None, ALU.mult)

        xa_mine = [persist.tile([P, L], dt.bfloat16, tag=f"xam{db}", name=f"xam{db}")
                   for db in range(4)]
        z16 = [persist.tile([P, L], dt.bfloat16, tag=f"z16_{m}", name=f"z16_{m}")
               for m in range(4)]
        spT0 = persist.tile([P, L], dt.float32, tag="spT0", name="spT0")
        sp16 = [persist.tile([P, L], dt.bfloat16, tag=f"sp16_{i}", name=f"sp16_{i}")
                for i in range(2)]
        cs16 = [persist.tile([P, L], dt.bfloat16, tag=f"cs16_{i}", name=f"cs16_{i}")
                for i in range(2)]
        y_acc = [persist.tile([P, L], dt.float32, tag=f"yacc{db}", name=f"yacc{db}")
                 for db in range(4)]

        sp16d = nc.dram_tensor("sp16_scratch", [2 * P, L], dt.bfloat16).ap()
        cs16d = nc.dram_tensor("cs16_scratch", [2 * P, L], dt.bfloat16).ap()

        # ================= pre phase =================
        with tc.tile_pool(name="stage", bufs=2) as stage, \
             tc.tile_pool(name="ps_mm", bufs=2, space=MS.PSUM) as ps_mm:
            # ---- LN1 (fp32 stats, bf16 output) ----
            cn = [stage.tile([P, L], dt.bfloat16, bufs=1, tag=f"cn{i}", name=f"cn{i}")
                  for i in range(4)]
            sn = [stage.tile([P, L], dt.bfloat16, bufs=1, tag=f"sn{i}", name=f"sn{i}")
                  for i in range(4)]
            with tc.tile_pool(name="ln_ps", bufs=1, space=MS.PSUM) as lnps:
                # input loads first on the sync queue: they gate the LN ->
                # style_proj / in_proj critical chains; weight DMAs follow.
                craw = [stage.tile([P, L], dt.float32, bufs=1, tag=f"craw{i}",
                                   name=f"craw{i}") for i in range(4)]
                for hi in range(4):
                    nc.sync.dma_start(craw[hi][:], din["sT"][hi * P:(hi + 1) * P, :])
                craw2 = [stage.tile([P, L], dt.float32, bufs=1, tag=f"craw2_{i}",
                                    name=f"craw2_{i}") for i in range(4)]
                for hi in range(4):
                    nc.sync.dma_start(craw2[hi][:], din["cT"][hi * P:(hi + 1) * P, :])
                n1w = load(wpool, "n1w")[0]
                n1b = load(wpool, "n1b")[0]
                w_cv = load(wpool, "w_conv")[0]
                cv_b = load(wpool, "conv_b")[0]
                w_dt = load(wpool, "w_dt")[0]
                dt_b = load(wpool, "dt_bias")[0]
                Ds = load(wpool, "Ds_sb")[0]
                w_op = load(wpool, "w_outproj")
                _layer_norm16(nc, stage, lnps, craw, sn, n1w, n1b, ones)
                _layer_norm16(nc, stage, lnps, craw2, cn, n1w, n1b, ones)
            emit_dbg("cn", cn)

            w_in = load(stage, "w_inproj")      # 4 x [128, 1536] bf16
            w_sp = load(stage, "w_styleproj")   # 4 x [128, 384] bf16
            w_cp = load(stage, "w_contentproj") # 8 x [128, 256] bf16

            # ---- style_proj: m=0 -> fp32 ranks; m=1,2 -> bf16 B rows ----
            for m in range(3):
                pt = ps_mm.tile([P, L], dt.float32, tag="mm_out")
                for hi in range(4):
                    for f in F2:
                        nc.tensor.matmul(pt[:, f],
                                         w_sp[hi][:, m * P:(m + 1) * P],
                                         sn[hi][:, f],
                                         start=(hi == 0), stop=(hi == 3))
                if m == 0:
                    nc.scalar.copy(spT0[:], pt[:])
                else:
                    nc.scalar.activation(sp16[m - 1][:], pt[:], AF.Copy)
                    nc.sync.dma_start(sp16d[(m - 1) * P:m * P, :], sp16[m - 1][:])

            # ---- per channel-block: in_proj -> pad -> dwconv -> silu -> Cs ----
            with tc.tile_pool(name="cs_ps", bufs=1, space=MS.PSUM) as csps:
                cs_pt = [csps.tile([P, L], dt.float32, tag=f"cs{m}", name=f"cs{m}")
                         for m in range(2)]
                for db in range(8):
                    pt = ps_mm.tile([P, L], dt.float32, tag="mm_out")
                    for hi in range(4):
                        for f in F2:
                            nc.tensor.matmul(pt[:, f],
                                             w_in[hi][:, db * P:(db + 1) * P],
                                             cn[hi][:, f],
                                             start=(hi == 0), stop=(hi == 3))
                    xpad = stage.tile([P, 34 * 34], dt.bfloat16, tag="xpad")
                    xp3 = xpad[:].rearrange("p (h w) -> p h w", h=34)
                    nc.vector.memset(xp3[:, 0:1, :], 0.0)
                    nc.vector.memset(xp3[:, 33:34, :], 0.0)
                    nc.vector.memset(xp3[:, 1:33, 0:1], 0.0)
                    nc.vector.memset(xp3[:, 1:33, 33:34], 0.0)
                    nc.scalar.activation(xp3[:, 1:33, 1:33],
                                         pt[:].rearrange("p (h w) -> p h w", h=32),
                                         AF.Copy)
                    cpt = ps_mm.tile([P, L], dt.float32, tag="mm_out")
                    for j in range(9):
                        dy, dx = j // 3, j % 3
                        dg = stage.tile([P, P], dt.bfloat16, tag="convdiag", bufs=3)
                        nc.scalar.activation(dg[:], ident16[:], AF.Copy,
                                             scale=w_cv[:, db * 9 + j:db * 9 + j + 1])
                        src = xp3[:, dy:dy + 32, dx:dx + 32]
                        for half in range(2):
                            nc.tensor.matmul(
                                cpt[:, half * 512:(half + 1) * 512], dg[:],
                                src[:, half * 16:(half + 1) * 16, :],
                                start=(j == 0), stop=(j == 8))
                    if db < 4:
                        xa = xa_mine[db]
                    else:
                        xa = stage.tile([P, L], dt.bfloat16, tag="xa_other")
                    sg = stage.tile([P, L], dt.bfloat16, bufs=2, tag="sil_s")
                    nc.scalar.activation(sg[:], cpt[:], AF.Sigmoid,
                                         bias=cv_b[:, db:db + 1])
                    xf = stage.tile([P, L], dt.bfloat16, bufs=2, tag="sil_x")
                    nc.scalar.activation(xf[:], cpt[:], AF.Identity,
                                         bias=cv_b[:, db:db + 1])
                    nc.vector.tensor_tensor(xa[:], xf[:], sg[:], ALU.mult)
                    for m in range(2):
                        for f in F2:
                            nc.tensor.matmul(cs_pt[m][:, f],
                                             w_cp[db][:, m * P:(m + 1) * P],
                                             xa[:, f],
                                             start=(db == 0), stop=(db == 7))
                for m in range(2):
                    nc.scalar.activation(cs16[m][:], cs_pt[m][:], AF.Copy)
                    nc.sync.dma_start(cs16d[m * P:(m + 1) * P, :], cs16[m][:])
            emit_dbg("xa", xa_mine)
            emit_dbg("cs16", cs16)
            emit_dbg("sp0", [spT0])
            emit_dbg("sp16", sp16)

            # ---- z part of in_proj (cols DI..DI+512 of host-sliced weight) ----
            for m in range(4):
                pt = ps_mm.tile([P, L], dt.float32, tag="mm_out")
                for hi in range(4):
                    for f in F2:
                        nc.tensor.matmul(pt[:, f],
                                         w_in[hi][:, (8 + m) * P:(9 + m) * P],
                                         cn[hi][:, f],
                                         start=(hi == 0), stop=(hi == 3))
                nc.scalar.activation(z16[m][:], pt[:], AF.Copy)
            emit_dbg("z", z16)

        # ================= scan phase =================
        with tc.tile_pool(name="dpool", bufs=1) as dpool, \
             tc.tile_pool(name="scan_sb", bufs=2) as scan_sb, \
             tc.tile_pool(name="bc_sb", bufs=2) as bc_sb, \
             tc.tile_pool(name="yps", bufs=1, space=MS.PSUM) as yps_pool:

            def bcast_src(dram, rowidx):
                rb = dram[rowidx:rowidx + 1, :]
                return bass.AP(rb.tensor, rb.offset, [[0, P]] + list(rb.ap[1:]))

            for k in range(K):
                yps = [yps_pool.tile([P, L], dt.float32, tag=f"yps{db}",
                                     name=f"yps{db}") for db in range(4)]
                delta = [dpool.tile([P, L], dt.bfloat16, tag=f"delta{db}", bufs=2,
                                    name=f"delta{db}") for db in range(4)]
                du = [dpool.tile([P, L], dt.bfloat16, tag=f"du{db}", bufs=2,
                                 name=f"du{db}") for db in range(4)]
                rank_k = scan_sb.tile([32, L], dt.float32, tag="rank_k")
                nc.sync.dma_start(rank_k[:], spT0[k * 32:(k + 1) * 32, :])
                for db in range(4):
                    for f in F2:
                        nc.tensor.matmul(yps[db][:, f],
                                         w_dt[:, k * 512 + db * P:
                                              k * 512 + (db + 1) * P],
                                         rank_k[:, f],
                                         start=True, stop=True)
                    dfp = scan_sb.tile([P, L], dt.float32, tag="dfp", bufs=2)
                    nc.scalar.activation(dfp[:], yps[db][:], AF.Exp,
                                         bias=dt_b[:, k * 4 + db:k * 4 + db + 1])
                    nc.scalar.activation(delta[db][:], dfp[:], AF.Ln,
                                         bias=ones[:, 0:1])
                    uv, is3d = _u_view(xa_mine[db], k)
                    if is3d:
                        nc.vector.tensor_tensor(
                            du[db][:].rearrange("p (a b) -> p a b", a=Hh),
                            delta[db][:].rearrange("p (a b) -> p a b", a=Hh),
                            uv, ALU.mult)
                    else:
                        nc.vector.tensor_tensor(du[db][:], delta[db][:], uv,
                                                ALU.mult)
                if "delta" in dbg and k == 0:
                    emit_dbg("delta", delta)

                Brow = (k // 2) * P + (k % 2) * 64
                Crow = (k // 2) * P + (k % 2) * 64
                # bcB depends only on the (early) style path; prefetch it with
                # a lookahead so the cs16d-gated bcC DMA (head-of-line on the
                # sync queue) does not stall dbu/scan work during the
                # conv/content_proj pre-phase.
                PRE = 10
                bcB_t = {}

                def fetch_bcB(i):
                    t = bc_sb.tile([P, L], dt.bfloat16, tag="bcB", bufs=12)
                    nc.sync.dma_start(t[:], bcast_src(sp16d, Brow + i))
                    bcB_t[i] = t

                for n in range(N):
                    if n == 0:
                        for i in range(PRE):
                            fetch_bcB(i)
                    if n + PRE < N:
                        fetch_bcB(n + PRE)
                    bcB = bcB_t.pop(n)
                    bcC = bc_sb.tile([P, L], dt.bfloat16, tag="bcC", bufs=6)
                    nc.sync.dma_start(bcC[:], bcast_src(cs16d, Crow + n))
                    for db in range(4):
                        acol = (k * 4 + db) * 64 + n
                        decay = scan_sb.tile([P, L], dt.bfloat16, tag="decay", bufs=6)
                        nc.scalar.activation(decay[:], delta[db][:], AF.Exp,
                                             scale=A_sb[:, acol:acol + 1])
                        dbu = scan_sb.tile([P, L], dt.bfloat16, tag="dbu", bufs=6)
                        nc.vector.tensor_tensor(dbu[:], du[db][:], bcB[:],
                                                ALU.mult)
                        h = scan_sb.tile([P, L], dt.bfloat16, tag="h", bufs=12)
                        nc.vector.tensor_tensor_scan(h[:], decay[:], dbu[:], 0.0,
                                                     ALU.mult, ALU.add)
                        hC = scan_sb.tile([P, L], dt.bfloat16, tag="hC", bufs=6)
                        eng = nc.vector if _dve_hc(k, n, db) else nc.gpsimd
                        eng.tensor_tensor(hC[:], h[:], bcC[:], ALU.mult)
                        for f in F2:
                            nc.tensor.matmul(yps[db][:, f], ident16[:],
                                             hC[:, f], start=(n == 0),
                                             stop=(n == N - 1))
                for db in range(4):
                    if k == 0:
                        nc.scalar.copy(y_acc[db][:], yps[db][:])
                    else:
                        nc.vector.scalar_tensor_tensor(
                            y_acc[db][:], yps[db][:], 1.0, y_acc[db][:],
                            ALU.mult, ALU.add)
                    uv, is3d = _u_view(xa_mine[db], k)
                    dscl = Ds[:, k * 4 + db:k * 4 + db + 1]
                    if is3d:
                        y3 = y_acc[db][:].rearrange("p (a b) -> p a b", a=Hh)
                        nc.vector.scalar_tensor_tensor(y3, uv, dscl, y3,
                                                       ALU.mult, ALU.add)
                    else:
                        nc.vector.scalar_tensor_tensor(
                            y_acc[db][:], uv, dscl, y_acc[db][:],
                            ALU.mult, ALU.add)
        emit_dbg("y", y_acc)

        # ================= out phase =================
        with tc.tile_pool(name="out_sb", bufs=2) as out_sb, \
             tc.tile_pool(name="out_ps", bufs=2, space=MS.PSUM) as out_ps:
            yz = []
            for db in range(4):
                t = out_sb.tile([P, L], dt.bfloat16, tag=f"yz{db}", bufs=1,
                                name=f"yz{db}")
                nc.vector.tensor_tensor(t[:], y_acc[db][:], z16[db][:], ALU.mult)
                yz.append(t)
            for m in range(4):
                pt = out_ps.tile([P, L], dt.float32, tag="mm_out")
                for db in range(4):
                    for f in F2:
                        nc.tensor.matmul(pt[:, f],
                                         w_op[db][:, m * P:(m + 1) * P],
                                         yz[db][:, f],
                                         start=(db == 0), stop=(db == 3))
                ot = out_sb.tile([P, L], dt.float32, tag="oev")
                nc.scalar.copy(ot[:], pt[:])
                nc.sync.dma_start(opart[m * P:(m + 1) * P, :], ot[:])
    return dbg_names


def build_launch2(nc):
    din = {name: nc.dram_tensor(name, shape, dty, kind="ExternalInput").ap()
           for name, shape, dty in L2_INPUTS}
    outT = nc.dram_tensor("outT", [HID, 512], dt.float32, kind="ExternalOutput").ap()
    T = 512

    ctx = ExitStack()
    with ctx:
        tc = ctx.enter_context(tile.TileContext(nc, trace_sim=False))
        wpool = ctx.enter_context(tc.tile_pool(name="wpool", bufs=1))
        sb = ctx.enter_context(tc.tile_pool(name="work", bufs=2))
        ps = ctx.enter_context(tc.tile_pool(name="psum", bufs=4, space=MS.PSUM))

        def load(name):
            src = din[name]
            nrows, fs = src.shape
            dty = src.tensor.dtype
            tiles = []
            for i in range(0, nrows, P):
                r = min(P, nrows - i)
                t = wpool.tile([r, fs], dty, tag=f"w_{name}_{i}",
                               name=f"ld_{name}_{i}")
                nc.sync.dma_start(t[:], src[i:i + r, :])
                tiles.append(t)
            return tiles

        # input loads first: they gate the LN -> MLP critical chain, while
        # the (much larger) weight DMAs are only needed once matmuls start.
        x1 = [wpool.tile([P, T], dt.float32, tag=f"x1_{i}", name=f"x1_{i}")
              for i in range(4)]
        xn = [wpool.tile([P, T], dt.bfloat16, tag=f"xn_{i}", name=f"xn_{i}")
              for i in range(4)]
        ld = []
        for hi in range(4):
            to = sb.tile([P, T], dt.float32, tag="ld_o")
            tr = sb.tile([P, T], dt.float32, tag="ld_r")
            nc.sync.dma_start(to[:], din["oT"][hi * P:(hi + 1) * P, :])
            nc.sync.dma_start(tr[:], din["rT"][hi * P:(hi + 1) * P, :])
            ld.append((to, tr))
        n2w = load("n2w")[0]
        n2b = load("n2b")[0]
        b1 = load("b_mlp1")[0]
        b2 = load("b_mlp2")[0]
        w1 = load("w_mlp1")
        w2 = load("w_mlp2")
        ones = wpool.tile([P, 2], dt.float32)
        nc.vector.memset(ones[:], 1.0)
        for hi in range(4):
            to, tr = ld[hi]
            nc.vector.tensor_tensor(x1[hi][:], to[:], tr[:], ALU.add)
        with tc.tile_pool(name="ln_ps", bufs=1, space=MS.PSUM) as lnps:
            _layer_norm16(nc, sb, lnps, x1, xn, n2w, n2b, ones, T=T)

        h1 = [wpool.tile([P, T], dt.bfloat16, tag=f"h1_{m}", name=f"h1_{m}")
              for m in range(16)]
        for m in range(16):
            pt = ps.tile([P, T], dt.float32, tag="mm_out")
            for hi in range(4):
                nc.tensor.matmul(pt[:], w1[hi][:, m * P:(m + 1) * P], xn[hi][:],
                                 start=(hi == 0), stop=(hi == 3))
            xb = sb.tile([P, T], dt.float32, tag="g_x")
            nc.scalar.activation(xb[:], pt[:], AF.Identity, bias=b1[:, m:m + 1])
            sq = sb.tile([P, T], dt.float32, tag="g_sq")
            nc.scalar.square(sq[:], xb[:])
            c0 = 0.7978845608028654
            nc.vector.tensor_scalar(sq[:], sq[:], 0.044715 * c0, c0,
                                    ALU.mult, ALU.add)
            nc.vector.tensor_tensor(sq[:], sq[:], xb[:], ALU.mult)
            th = sb.tile([P, T], dt.float32, tag="g_th")
            nc.scalar.activation(th[:], sq[:], AF.Tanh)
            nc.vector.tensor_scalar(th[:], th[:], 0.5, 0.5, ALU.mult, ALU.add)
            nc.vector.tensor_tensor(h1[m][:], th[:], xb[:], ALU.mult)

        for m in range(4):
            pt = ps.tile([P, T], dt.float32, tag="mm_out")
            for mi in range(16):
                nc.tensor.matmul(pt[:], w2[mi][:, m * P:(m + 1) * P], h1[mi][:],
                                 start=(mi == 0), stop=(mi == 15))
            ot = sb.tile([P, T], dt.float32, tag="oev")
            nc.scalar.activation(ot[:], pt[:], AF.Identity, bias=b2[:, m:m + 1])
            nc.vector.tensor_tensor(ot[:], ot[:], x1[m][:], ALU.add)
            nc.sync.dma_start(outT[m * P:(m + 1) * P, :], ot[:])
    return []


# ---------------- host-side prep ----------------

def _perm(dh):
    """DI permutation putting this core's d-half first."""
    p = np.arange(DI)
    return np.concatenate([p[dh * 512:dh * 512 + 512],
                           p[(1 - dh) * 512:(1 - dh) * 512 + 512]])


def prep_launch1(inputs, c):
    b, dh = c // 2, c % 2
    dsl = slice(dh * 512, dh * 512 + 512)
    perm = _perm(dh)
    f32 = np.float32
    A = lambda x: np.ascontiguousarray(x, f32)
    B16 = lambda x: np.ascontiguousarray(np.asarray(x, f32).astype(BF16))
    d = {}
    d["cT"] = A(np.asarray(inputs["content"])[b].reshape(L, HID).T)
    d["sT"] = A(np.asarray(inputs["style"])[b].reshape(L, HID).T)
    w_in = np.asarray(inputs["in_proj_w"])
    d["w_inproj"] = B16(np.concatenate(
        [w_in[:, :DI][:, perm], w_in[:, DI:][:, dsl]], axis=1))
    d["w_conv"] = A(np.asarray(inputs["conv_w"]).reshape(DI, 9)[perm]
                    .reshape(8, P, 9).transpose(1, 0, 2).reshape(P, 72))
    d["conv_b"] = A(np.asarray(inputs["conv_b"])[perm].reshape(8, P).T)
    d["w_styleproj"] = B16(inputs["style_proj_w"])
    d["w_contentproj"] = B16(np.asarray(inputs["content_proj_w"])[perm])
    d["w_dt"] = A(np.concatenate(
        [np.asarray(inputs["dt_projs_weight"])[k, dsl, :].T for k in range(K)],
        axis=1))
    d["dt_bias"] = A(np.asarray(inputs["dt_projs_bias"])[:, dsl]
                     .reshape(K * 4, P).T)
    d["A_logs_sb"] = A(np.concatenate(
        [np.asarray(inputs["A_logs"]).reshape(K, DI, N)[k, dsl][db * P:(db + 1) * P]
         for k in range(K) for db in range(4)], axis=1))
    d["Ds_sb"] = A(np.asarray(inputs["Ds"]).reshape(K, DI)[:, dsl]
                   .reshape(K * 4, P).T)
    d["w_outproj"] = B16(np.asarray(inputs["out_proj_w"])[dsl, :])
    d["n1w"] = A(np.asarray(inputs["norm1_w"]).reshape(4, P).T)
    d["n1b"] = A(np.asarray(inputs["norm1_b"]).reshape(4, P).T)
    return d


def prep_launch2(inputs, oT_full, c):
    b, th = c // 2, c % 2
    tsl = slice(th * 512, th * 512 + 512)
    f32 = np.float32
    A = lambda x: np.ascontiguousarray(x, f32)
    B16 = lambda x: np.ascontiguousarray(np.asarray(x, f32).astype(BF16))
    d = {}
    d["oT"] = A(oT_full[b][:, tsl])
    d["rT"] = A(np.asarray(inputs["content"])[b].reshape(L, HID).T[:, tsl])
    d["w_mlp1"] = B16(inputs["mlp_w1"])
    d["b_mlp1"] = A(np.asarray(inputs["mlp_b1"]).reshape(16, P).T)
    d["w_mlp2"] = B16(inputs["mlp_w2"])
    d["b_mlp2"] = A(np.asarray(inputs["mlp_b2"]).reshape(4, P).T)
    d["n2w"] = A(np.asarray(inputs["norm2_w"]).reshape(4, P).T)
    d["n2b"] = A(np.asarray(inputs["norm2_b"]).reshape(4, P).T)
    return d


# ---------------- self-contained kernel entry ----------------
import concourse.bacc as _bacc
from concourse.bass_utils import run_bass_kernel_spmd as _run_spmd

_CACHE = {}
LAST_EXEC_NS = None


def _get_nc(which):
    if which in _CACHE:
        return _CACHE[which]
    nc = _bacc.Bacc("TRN2", target_bir_lowering=False, debug=False)
    (build_launch1 if which == 1 else build_launch2)(nc)
    nc.compile()
    _CACHE[which] = nc
    return nc


def kernel(**inputs):
    inputs = {k: np.asarray(v) for k, v in inputs.items()}
    nc1 = _get_nc(1)
    in_maps1 = [prep_launch1(inputs, c) for c in range(8)]
    res1 = _run_spmd(nc1, in_maps1, core_ids=list(range(8)))
    oT_full = [res1.results[2 * b]["opart"].astype(np.float32)
               + res1.results[2 * b + 1]["opart"].astype(np.float32)
               for b in range(Bb)]
    nc2 = _get_nc(2)
    in_maps2 = [prep_launch2(inputs, oT_full, c) for c in range(8)]
    res2 = _run_spmd(nc2, in_maps2, core_ids=list(range(8)))
    out = np.zeros((Bb, Hh, Ww, HID), np.float32)
    for c in range(8):
        b, th = c // 2, c % 2
        out[b].reshape(L, HID)[th * 512:(th + 1) * 512, :] = \
            res2.results[c]["outT"].T
    return out
